# Initial kernel scaffold
#
"""Axial attention block (B=8, C=512, H=W=128, 8 heads) on 8 Trainium2 cores.

Sharding: data-parallel over batch — one batch element per NeuronCore. Each
core runs both axial passes on its (C, H, W) slice and produces the full
residual sum xs + oh + ow.

Pass structure (all DMA contiguous-run; no strided DRAM access):
  - Pass 1 (HEIGHT attention, sequences along h, one per w): reads xtbf
    (C,W,H) chunks, computes oh tiles in (c, w-chunk, h) layout and writes
    them to a block-tiled DRAM scratch ohT2[hb][c, w, hi] (h = hb*8 + hi).
    The SBUF stage tile is laid out (hb, w, hi) so both DMA sides have
    >=32B/512B contiguous runs.
  - Pass 2 (WIDTH attention, sequences along w, one per h): h-chunk hb reads
    xbf + xs(f32) chunks and the matching ohT2[hb] block (contiguous), folds
    oh into the f32 residual once per chunk (GpSimd), then out = ow + resid
    per group (VectorE) and writes natural-layout output.

Matmul inputs are pre-cast to bf16 on the host (xbf natural, xtbf h<->w
swapped); weights pre-transposed to (c_in, c_out) bf16.

Per-sequence attention (S=128, dh=64): scoresT = K^T.T @ Q^T per head in
(s_k, s_q) layout, parity-split over two PSUM banks (concurrent row-group
matmuls must not share a bank); exp on ScalarE (max-subtraction skipped —
scaled scores are bounded ~7); denominators via ones-matmul on TensorE
(replicated rows); reciprocal_approx_fast + normalize on VectorE; AV packs
all 8 heads into one PSUM bank in (c, s_q) layout; O-projection batched
over 4 sequences.
"""
import os
import numpy as np
import ml_dtypes

P = 128          # partitions
C = 512          # channels
S = 128          # sequence length (H and W)
NCB = C // P     # channel blocks
NH = 8           # heads
DH = C // NH     # head dim
G = 4            # sequences per projection group
HC1 = 16         # w-chunk, height pass
HC2 = 8          # h-chunk, width pass (= hi block size of ohT2)
HB = S // HC2    # number of h blocks
NCORES = 8

_BF16 = ml_dtypes.bfloat16

# schedule-tuning knobs (env-overridable for experiments)
PROJ_BUFS = int(os.environ.get("K_PROJ_BUFS", "2"))
ATTN_BUFS = int(os.environ.get("K_ATTN_BUFS", "2"))
ET_BUFS = int(os.environ.get("K_ET_BUFS", "2"))
QK_BUFS = int(os.environ.get("K_QK_BUFS", "2"))
VT_BUFS = int(os.environ.get("K_VT_BUFS", "2"))
OT_BUFS = int(os.environ.get("K_OT_BUFS", "2"))
RR_BUFS = int(os.environ.get("K_RR_BUFS", "2"))
PO_BUFS = int(os.environ.get("K_PO_BUFS", "2"))
VT_ON_ACT = int(os.environ.get("K_VT_ON_ACT", "0"))
STG1_ON_ACT = int(os.environ.get("K_STG1_ON_ACT", "0"))
QT_ENG = os.environ.get("K_QT_ENG", "act")
KT_ENG = os.environ.get("K_KT_ENG", "act")

_PROG = None  # cached compiled Bass program


def _build_program():
    from contextlib import ExitStack
    import concourse.tile as tile
    from concourse import bacc, mybir

    f32 = mybir.dt.float32
    bf = mybir.dt.bfloat16
    Exp = mybir.ActivationFunctionType.Exp

    nc = bacc.Bacc("TRN2", target_bir_lowering=False, debug=False)

    xf = nc.dram_tensor("xf", [C, S, S], f32, kind="ExternalInput").ap()
    xbf = nc.dram_tensor("xbf", [C, S, S], bf, kind="ExternalInput").ap()
    xtbf = nc.dram_tensor("xtbf", [C, S, S], bf, kind="ExternalInput").ap()
    wnames = ["wq_w", "wk_w", "wv_w", "wo_w", "wq_h", "wk_h", "wv_h", "wo_h"]
    wt = {n: nc.dram_tensor(n, [C, C], bf, kind="ExternalInput").ap() for n in wnames}
    ohT2 = nc.dram_tensor("ohT2", [HB, C, S, HC2], f32).ap()
    out = nc.dram_tensor("out", [C, S, S], f32, kind="ExternalOutput").ap()

    with tile.TileContext(nc) as tc, ExitStack() as topctx:
        const = topctx.enter_context(tc.tile_pool(name="const", bufs=1))

        w_sb = {}
        for n in wnames:
            tiles = []
            for ci in range(NCB):
                t = const.tile([P, C], bf, tag=f"w_{n}_{ci}", name=f"w_{n}_{ci}")
                nc.sync.dma_start(out=t, in_=wt[n][ci * P:(ci + 1) * P, :])
                tiles.append(t)
            w_sb[n] = tiles
        ones_sb = const.tile([P, P], bf, tag="ones", name="ones")
        nc.vector.memset(ones_sb, 1.0)

        def attn_group(src_t, gsl, s0, wq, wk, wv, wo, pools):
            """One group of G sequences -> psum tiles of out-projection
            results, one (P, G*S) tile per c_out block."""
            qk_pool, vt_pool, ot_pool, et_pool, rr_pool, proj_ps, attn_ps, po_ps = pools

            qt_sb, kt_sb = [], []
            for wmat, dst_list, nm in ((wq, qt_sb, "qt"), (wk, kt_sb, "kt")):
                for co in range(NCB):
                    pp = proj_ps.tile([P, G * S], f32, tag="proj", name="pp")
                    for ci in range(NCB):
                        nc.tensor.matmul(
                            pp,
                            lhsT=wmat[ci][:, co * P:(co + 1) * P],
                            rhs=src_t[ci][:, gsl, :],
                            start=(ci == 0), stop=(ci == NCB - 1))
                    sb_t = qk_pool.tile([P, G * S], bf, tag=f"{nm}{co}", name=f"{nm}{co}")
                    if (nm == "qt" and QT_ENG == "dve") or (nm == "kt" and KT_ENG == "dve"):
                        nc.vector.tensor_copy(sb_t, pp)
                    else:
                        nc.scalar.copy(sb_t, pp)
                    dst_list.append(sb_t)

            vt_sb = []
            for sq in range(G):
                pv = proj_ps.tile([P, C], f32, tag="proj", name="pv")
                for ci in range(NCB):
                    nc.tensor.matmul(
                        pv, lhsT=src_t[ci][:, s0 + sq, :], rhs=wv[ci],
                        start=(ci == 0), stop=(ci == NCB - 1))
                vt = vt_pool.tile([P, C], bf, tag=f"vt{sq}", name=f"vt{sq}")
                if VT_ON_ACT:
                    nc.scalar.copy(vt, pv)
                else:
                    nc.vector.tensor_copy(vt, pv)
                vt_sb.append(vt)

            ot_full = ot_pool.tile([P, NCB, G * S], bf, tag="ot", name="ot")
            for sq in range(G):
                ssl = slice(sq * S, (sq + 1) * S)
                # scoresT: head h -> col h//2*128 of half (h%2); the two
                # 512-col halves are separate PSUM banks, so even (row-group
                # 0-63) and odd (64-127) head matmuls never share a bank
                st2 = attn_ps.tile([P, 1024], f32, tag="attn", name="st2")
                for h in range(NH):
                    par, cb = h % 2, h // 2
                    rows = slice(par * DH, (par + 1) * DH)
                    nc.tensor.matmul(
                        st2[:, par * 512 + cb * S:par * 512 + (cb + 1) * S],
                        lhsT=kt_sb[h // 2][rows, ssl],
                        rhs=qt_sb[h // 2][rows, ssl],
                        start=True, stop=True)
                et = et_pool.tile([P, 1024], bf, tag="et", name="et")
                nc.scalar.activation(out=et, in_=st2, func=Exp, scale=DH ** -0.5)
                r2 = attn_ps.tile([P, 1024], f32, tag="attn", name="r2")
                nc.tensor.matmul(r2[:, 0:512], lhsT=ones_sb, rhs=et[:, 0:512],
                                 start=True, stop=True)
                nc.tensor.matmul(r2[:, 512:1024], lhsT=ones_sb, rhs=et[:, 512:1024],
                                 start=True, stop=True)
                rr = rr_pool.tile([P, 1024], f32, tag="rr", name="rr")
                nc.vector.reciprocal_approx_fast(out=rr, in_=r2)
                # AV on unnormalized exp; softmax denominators commute past
                # the matmul (pure column scaling), so recip runs on VectorE
                # in parallel with AV on TensorE and the normalize fuses into
                # the psum->sbuf evacuation below.
                po = po_ps.tile([P, 512], f32, tag="po", name="po")
                for h in range(NH):
                    par, cb = h % 2, h // 2
                    nc.tensor.matmul(
                        po[par * DH:(par + 1) * DH, cb * S:(cb + 1) * S],
                        lhsT=vt_sb[sq][:, h * DH:(h + 1) * DH],
                        rhs=et[:, par * 512 + cb * S:par * 512 + (cb + 1) * S],
                        start=True, stop=True)
                # row-half r of po holds heads with parity r; its per-element
                # normalizer is exactly rr[:, r*512:] (rows replicated)
                nc.vector.tensor_mul(
                    ot_full[0:DH, :, ssl],
                    po[0:DH, :].rearrange("p (c s) -> p c s", c=NCB),
                    rr[0:DH, 0:512].rearrange("p (c s) -> p c s", c=NCB))
                nc.vector.tensor_mul(
                    ot_full[DH:P, :, ssl],
                    po[DH:P, :].rearrange("p (c s) -> p c s", c=NCB),
                    rr[DH:P, 512:1024].rearrange("p (c s) -> p c s", c=NCB))
            # O-projection outputs go through the attn pool's 2-bank tiles
            # (pairs of c_out blocks in the two bank halves) so the proj pool
            # frees up for the next group's Q/K/V immediately
            pods = []
            for cop in range(NCB // 2):
                pp2 = attn_ps.tile([P, 1024], f32, tag="attn", name="pp2")
                for half in range(2):
                    co = cop * 2 + half
                    dst = pp2[:, half * 512:(half + 1) * 512]
                    for ci in range(NCB):
                        nc.tensor.matmul(
                            dst,
                            lhsT=wo[ci][:, co * P:(co + 1) * P],
                            rhs=ot_full[:, ci, :],
                            start=(ci == 0), stop=(ci == NCB - 1))
                    pods.append(dst)
            return pods

        def height_pass():
            """Pass 1: height attention (seq along h, one per w).  Writes oh
            to the blocked scratch ohT2[hb][c, w, hi]."""
            wq, wk, wv, wo = (w_sb["wq_h"], w_sb["wk_h"], w_sb["wv_h"], w_sb["wo_h"])
            with ExitStack() as ctx:
                src_pool = ctx.enter_context(tc.tile_pool(name="src1", bufs=2))
                stage_pool = ctx.enter_context(tc.tile_pool(name="stg1", bufs=2))
                qk_pool = ctx.enter_context(tc.tile_pool(name="qk1", bufs=QK_BUFS))
                vt_pool = ctx.enter_context(tc.tile_pool(name="vt1", bufs=VT_BUFS))
                ot_pool = ctx.enter_context(tc.tile_pool(name="ot1", bufs=OT_BUFS))
                et_pool = ctx.enter_context(tc.tile_pool(name="et1", bufs=ET_BUFS))
                rr_pool = ctx.enter_context(tc.tile_pool(name="rr1", bufs=RR_BUFS))
                proj_ps = ctx.enter_context(tc.tile_pool(name="pps1", bufs=PROJ_BUFS, space="PSUM"))
                attn_ps = ctx.enter_context(tc.tile_pool(name="aps1", bufs=ATTN_BUFS, space="PSUM"))
                po_ps = ctx.enter_context(tc.tile_pool(name="pops1", bufs=PO_BUFS, space="PSUM"))
                pools = (qk_pool, vt_pool, ot_pool, et_pool, rr_pool, proj_ps, attn_ps, po_ps)

                for chunk in range(S // HC1):
                    q0 = chunk * HC1
                    src_t, stage_t = [], []
                    for cb in range(NCB):
                        cs = slice(cb * P, (cb + 1) * P)
                        t = src_pool.tile([P, HC1, S], bf, tag=f"src{cb}", name=f"src{cb}")
                        nc.sync.dma_start(out=t, in_=xtbf[cs, q0:q0 + HC1, :])
                        src_t.append(t)
                        # stage layout (hb, w, hi): contiguous runs on both
                        # DMA sides of the blocked write
                        st = stage_pool.tile([P, HB, HC1, HC2], f32, tag=f"stg{cb}", name=f"stg{cb}")
                        stage_t.append(st)
                    for g in range(HC1 // G):
                        s0 = g * G
                        gsl = slice(s0, s0 + G)
                        pods = attn_group(src_t, gsl, s0, wq, wk, wv, wo, pools)
                        for co in range(NCB):
                            # pods: (p, 4 w-seq, 128 h) -> stage (hb, w in gsl, hi)
                            stage_op = nc.scalar.copy if STG1_ON_ACT else nc.vector.tensor_copy
                            stage_op(
                                stage_t[co][:, :, gsl, :].rearrange("p b q i -> p q b i"),
                                pods[co].rearrange("p (q b i) -> p q b i", q=G, b=HB))
                    for cb in range(NCB):
                        cs = slice(cb * P, (cb + 1) * P)
                        nc.sync.dma_start(
                            out=ohT2[:, cs, q0:q0 + HC1, :].rearrange("b c w i -> c b w i"),
                            in_=stage_t[cb])

        def width_pass():
            """Pass 2: width attention (seq along w, one per h).  h-chunk =
            hb block; out = xs + oh + ow in natural layout."""
            wq, wk, wv, wo = (w_sb["wq_w"], w_sb["wk_w"], w_sb["wv_w"], w_sb["wo_w"])
            with ExitStack() as ctx:
                src_pool = ctx.enter_context(tc.tile_pool(name="src2", bufs=2))
                resid_pool = ctx.enter_context(tc.tile_pool(name="res2", bufs=2))
                oh_pool = ctx.enter_context(tc.tile_pool(name="oh2", bufs=2))
                stage_pool = ctx.enter_context(tc.tile_pool(name="stg2", bufs=2))
                qk_pool = ctx.enter_context(tc.tile_pool(name="qk2", bufs=QK_BUFS))
                vt_pool = ctx.enter_context(tc.tile_pool(name="vt2", bufs=VT_BUFS))
                ot_pool = ctx.enter_context(tc.tile_pool(name="ot2", bufs=OT_BUFS))
                et_pool = ctx.enter_context(tc.tile_pool(name="et2", bufs=ET_BUFS))
                rr_pool = ctx.enter_context(tc.tile_pool(name="rr2", bufs=RR_BUFS))
                proj_ps = ctx.enter_context(tc.tile_pool(name="pps2", bufs=PROJ_BUFS, space="PSUM"))
                attn_ps = ctx.enter_context(tc.tile_pool(name="aps2", bufs=ATTN_BUFS, space="PSUM"))
                po_ps = ctx.enter_context(tc.tile_pool(name="pops2", bufs=PO_BUFS, space="PSUM"))
                pools = (qk_pool, vt_pool, ot_pool, et_pool, rr_pool, proj_ps, attn_ps, po_ps)

                for hb in range(HB):
                    q0 = hb * HC2
                    src_t, resid_t, stage_t = [], [], []
                    for cb in range(NCB):
                        cs = slice(cb * P, (cb + 1) * P)
                        t = src_pool.tile([P, HC2, S], bf, tag=f"src{cb}", name=f"src{cb}")
                        nc.sync.dma_start(out=t, in_=xbf[cs, q0:q0 + HC2, :])
                        src_t.append(t)
                        rt = resid_pool.tile([P, HC2, S], f32, tag=f"res{cb}", name=f"res{cb}")
                        nc.sync.dma_start(out=rt, in_=xf[cs, q0:q0 + HC2, :])
                        resid_t.append(rt)
                        oht = oh_pool.tile([P, S, HC2], f32, tag=f"oh{cb}", name=f"oh{cb}")
                        nc.sync.dma_start(out=oht, in_=ohT2[hb, cs, :, :])
                        # fold oh into the residual once per chunk
                        nc.gpsimd.tensor_tensor(
                            out=rt, in0=rt,
                            in1=oht.rearrange("p w i -> p i w"),
                            op=mybir.AluOpType.add)
                        st = stage_pool.tile([P, HC2, S], f32, tag=f"stg{cb}", name=f"stg{cb}")
                        stage_t.append(st)
                    for g in range(HC2 // G):
                        s0 = g * G
                        gsl = slice(s0, s0 + G)
                        pods = attn_group(src_t, gsl, s0, wq, wk, wv, wo, pools)
                        for co in range(NCB):
                            nc.vector.tensor_add(
                                stage_t[co][:, gsl, :],
                                pods[co].rearrange("p (q s) -> p q s", q=G),
                                resid_t[co][:, gsl, :])
                    for cb in range(NCB):
                        cs = slice(cb * P, (cb + 1) * P)
                        nc.sync.dma_start(out=out[cs, q0:q0 + HC2, :], in_=stage_t[cb])

        height_pass()
        width_pass()

    nc.compile()
    return nc


def _get_program():
    global _PROG
    if _PROG is None:
        _PROG = _build_program()
    return _PROG


def kernel(xs, Wq_h, Wk_h, Wv_h, Wo_h, Wq_w, Wk_w, Wv_w, Wo_w):
    from concourse.bass_utils import run_bass_kernel_spmd

    nc = _get_program()

    wmap = {
        "wq_w": Wq_w, "wk_w": Wk_w, "wv_w": Wv_w, "wo_w": Wo_w,
        "wq_h": Wq_h, "wk_h": Wk_h, "wv_h": Wv_h, "wo_h": Wo_h,
    }
    wt_np = {n: np.ascontiguousarray(np.asarray(w, dtype=np.float32).T).astype(_BF16)
             for n, w in wmap.items()}

    xs = np.asarray(xs, dtype=np.float32)
    in_maps = []
    for b in range(NCORES):
        xb = np.ascontiguousarray(xs[b])                        # (C, H, W) f32
        xbf = xb.astype(_BF16)                                  # (C, H, W) bf16
        xtbf = np.ascontiguousarray(np.swapaxes(xb, 1, 2)).astype(_BF16)  # (C, W, H)
        in_maps.append({"xf": xb, "xbf": xbf, "xtbf": xtbf, **wt_np})

    res = run_bass_kernel_spmd(nc, in_maps, core_ids=list(range(NCORES)))
    return np.stack([res.results[b]["out"] for b in range(NCORES)], axis=0)



# revision 3
# speedup vs baseline: 1.0035x; 1.0035x over previous
"""Axial attention block (B=8, C=512, H=W=128, 8 heads) on 8 Trainium2 cores.

v3: fp8e4m3 DoubleRow V/O projections, fused softmax denominators, exp
overlap, group-level software pipelining (next group's Q/K projections are
emitted between this group's last AV and its O-projection, so the ScalarE
copy burst overlaps TensorE work without delaying this group's exps), and
the width-pass residual added on TensorE via an identity matmul accumulated
into the O-projection psum group.  Sharding: data-parallel over batch — one
batch element per core.

Precision: rel err ~1.3e-2 vs the f32 reference (fp8 V/O path + bf16
elsewhere); Q/K projections stay bf16 because fp8 there amplifies through
the exp.
"""
import os
import numpy as np
import ml_dtypes

P = 128          # partitions
C = 512          # channels
S = 128          # sequence length (H and W)
NCB = C // P     # channel blocks
NPR = NCB // 2   # channel pair-blocks (fp8 DoubleRow K=256)
NH = 8           # heads
DH = C // NH     # head dim
G = 4            # sequences per projection group
HC1 = 16         # w-chunk, height pass
HC2 = 8          # h-chunk, width pass (= hi block size of ohT2)
HB = S // HC2    # number of h blocks
NCORES = 8

_BF16 = ml_dtypes.bfloat16
_F8 = ml_dtypes.float8_e4m3
_IDM = np.eye(128, dtype=np.float32).astype(_BF16)

# build-time flags (env-overridable for experiments)
K_QK8 = int(os.environ.get("K_QK8", "0"))      # fp8 Q/K projections
K_V8 = int(os.environ.get("K_V8", "1"))        # fp8 V projection
K_O8 = int(os.environ.get("K_O8", "1"))        # fp8 ot + O-proj
# engine assignment knobs: "act" or "dve"
K_SCATTER_ENG = os.environ.get("K_SCATTER_ENG", "act")
K_QK_ENG = os.environ.get("K_QK_ENG", "act")
K_STGH_ENG = os.environ.get("K_STGH_ENG", "dve")
K_STGW_ENG = os.environ.get("K_STGW_ENG", "dve")
K_QK_BUFS = int(os.environ.get("K_QK_BUFS", "2"))
K_ET_BUFS = int(os.environ.get("K_ET_BUFS", "3"))
K_RR_BUFS = int(os.environ.get("K_RR_BUFS", "2"))
K_OT_BUFS = int(os.environ.get("K_OT_BUFS", "2"))

_NEED_F8_SRC = K_QK8 or K_V8
_NEED_BF_SRC = (not K_QK8) or (not K_V8)

_PROG = {}  # reps -> compiled Bass program


def _build_program(reps=1):
    from contextlib import ExitStack
    import concourse.tile as tile
    from concourse import bacc, mybir

    f32 = mybir.dt.float32
    bf = mybir.dt.bfloat16
    f8 = mybir.dt.float8e4
    DR = mybir.MatmulPerfMode.DoubleRow
    Exp = mybir.ActivationFunctionType.Exp

    odt = f8 if K_O8 else bf

    nc = bacc.Bacc("TRN2", target_bir_lowering=False, debug=False)

    xf = nc.dram_tensor("xf", [C, S, S], f32, kind="ExternalInput").ap()
    xn = nc.dram_tensor("xn", [C, S, S], bf, kind="ExternalInput").ap()
    xt = nc.dram_tensor("xt", [C, S, S], bf, kind="ExternalInput").ap()
    xn8 = nc.dram_tensor("xn8", [C, S, S], f8, kind="ExternalInput").ap()
    xt8 = nc.dram_tensor("xt8", [C, S, S], f8, kind="ExternalInput").ap()
    idm = nc.dram_tensor("idm", [P, P], bf, kind="ExternalInput").ap()
    wnames = ["wq_w", "wk_w", "wv_w", "wo_w", "wq_h", "wk_h", "wv_h", "wo_h"]

    def w_is_f8(n):
        if n.startswith("wo"):
            return K_O8
        if n.startswith("wv"):
            return K_V8
        return K_QK8

    wt = {n: nc.dram_tensor(n, [C, C], f8 if w_is_f8(n) else bf,
                            kind="ExternalInput").ap() for n in wnames}
    ohT2 = nc.dram_tensor("ohT2", [HB, C, S, HC2], bf).ap()
    out = nc.dram_tensor("out", [C, S, S], f32, kind="ExternalOutput").ap()

    with tile.TileContext(nc) as tc, ExitStack() as topctx:
        const = topctx.enter_context(tc.tile_pool(name="const", bufs=1))

        # weights in SBUF.  fp8 weights use the DoubleRow pair layout
        # [p, i, co] = W[pair*256 + i*128 + p, co]; bf16 keep [p, co] per ci.
        w_sb = {}
        for n in wnames:
            tiles = []
            if w_is_f8(n):
                for j in range(NPR):
                    t = const.tile([P, 2, C], f8, tag=f"w_{n}_{j}", name=f"w_{n}_{j}")
                    nc.sync.dma_start(
                        out=t,
                        in_=wt[n][j * 2 * P:(j + 1) * 2 * P, :].rearrange(
                            "(i p) co -> p i co", i=2))
                    tiles.append(t)
            else:
                for ci in range(NCB):
                    t = const.tile([P, C], bf, tag=f"w_{n}_{ci}", name=f"w_{n}_{ci}")
                    nc.sync.dma_start(out=t, in_=wt[n][ci * P:(ci + 1) * P, :])
                    tiles.append(t)
            w_sb[n] = tiles

        ident = const.tile([P, P], bf, tag="ident", name="ident")
        nc.sync.dma_start(out=ident, in_=idm)

        def copy_on(eng, out_, in_):
            (nc.scalar.copy if eng == "act" else nc.vector.tensor_copy)(out_, in_)

        # vt buffers: [128, NH*128] bf16; head block h holds [V_h | 1] for
        # even h, [1 | V_h] for odd h (ones cols memset once, V cols
        # rewritten per sequence).  2*G buffers: groups alternate banks so
        # group g+1's V scatter never waits on group g's AV reads.
        vt_bufs = []
        for b in range(2 * G):
            vb = const.tile([P, NH * P], bf, tag=f"vt{b}", name=f"vt{b}")
            nc.vector.memset(vb, 1.0)
            vt_bufs.append(vb)

        def mk_pools(ctx, tag):
            pools = {}
            pools["qk"] = ctx.enter_context(tc.tile_pool(name=f"qk{tag}", bufs=K_QK_BUFS))
            pools["ot"] = ctx.enter_context(tc.tile_pool(name=f"ot{tag}", bufs=K_OT_BUFS))
            pools["et"] = ctx.enter_context(tc.tile_pool(name=f"et{tag}", bufs=K_ET_BUFS))
            pools["rr"] = ctx.enter_context(tc.tile_pool(name=f"rr{tag}", bufs=K_RR_BUFS))
            pools["proj"] = ctx.enter_context(
                tc.tile_pool(name=f"pps{tag}", bufs=2, space="PSUM"))
            pools["attn"] = ctx.enter_context(
                tc.tile_pool(name=f"aps{tag}", bufs=3, space="PSUM"))
            return pools

        def qk_phase(src_bf, src_f8, gsl, wq, wk, pools):
            """Q/K projections -> qt/kt sbuf tiles (c_out part, s_q free)."""
            qt_sb, kt_sb = [], []
            for wmat, dst_list, nm in ((wq, qt_sb, "qt"), (wk, kt_sb, "kt")):
                for co in range(NCB):
                    pp = pools["proj"].tile([P, G * S], f32, tag="proj", name="pp")
                    if K_QK8:
                        for j in range(NPR):
                            nc.tensor.matmul(
                                pp, lhsT=wmat[j][:, :, co * P:(co + 1) * P],
                                rhs=src_f8[j][:, :, gsl, :],
                                start=(j == 0), stop=(j == NPR - 1), perf_mode=DR)
                    else:
                        for ci in range(NCB):
                            nc.tensor.matmul(
                                pp, lhsT=wmat[ci][:, co * P:(co + 1) * P],
                                rhs=src_bf[ci][:, gsl, :],
                                start=(ci == 0), stop=(ci == NCB - 1))
                    sb_t = pools["qk"].tile([P, G * S], bf, tag=f"{nm}{co}",
                                            name=f"{nm}{co}")
                    copy_on(K_QK_ENG, sb_t, pp)
                    dst_list.append(sb_t)
            return qt_sb, kt_sb

        def attn_body(qtkt, src_bf, src_f8, s0, wv, wo, pools, gen,
                      resid=None, next_qk=None):
            """scores/exp/AV/normalize for one group + O-projection; emits
            next_qk() (the following group's Q/K phase) between the last AV
            and the O-projection so its PE+ACT burst overlaps this group's
            tail without delaying the exps."""
            qt_sb, kt_sb = qtkt
            gsl = slice(s0, s0 + G)
            vslot = (gen % 2) * G

            def scores_seq(sq):
                """scoresT (s_k, s_q); head h -> cols par*512 + cb*128
                (par=h%2, cb=h//2): even/odd heads in different PSUM banks."""
                ssl = slice(sq * S, (sq + 1) * S)
                st2 = pools["attn"].tile([P, 1024], f32, tag="attn", name="st2")
                for h in range(NH):
                    par, cb = h % 2, h // 2
                    rows = slice(par * DH, (par + 1) * DH)
                    nc.tensor.matmul(
                        st2[:, par * 512 + cb * S:par * 512 + (cb + 1) * S],
                        lhsT=kt_sb[h // 2][rows, ssl],
                        rhs=qt_sb[h // 2][rows, ssl],
                        start=True, stop=True)
                et = pools["et"].tile([P, 1024], bf, tag="et", name="et")
                nc.scalar.activation(out=et, in_=st2, func=Exp, scale=DH ** -0.5)
                return et

            def et_block(et, h):
                par, cb = h % 2, h // 2
                return et[:, par * 512 + cb * S:par * 512 + (cb + 1) * S]

            ot_full = pools["ot"].tile([P, NCB, G * S], odt, tag="ot", name="ot")

            def attn_tail(sq, et):
                """AV with embedded ones -> denominators in the opposite row
                half; one full recip; partition-crossing normalize."""
                ssl = slice(sq * S, (sq + 1) * S)
                vt = vt_bufs[vslot + sq]
                po = pools["attn"].tile([P, 1024], f32, tag="attn", name="po")
                for h in (0, 4, 1, 5, 2, 6, 3, 7):  # alternate PSUM banks
                    nc.tensor.matmul(
                        po[:, h * P:(h + 1) * P],
                        lhsT=vt[:, h * P:(h + 1) * P],
                        rhs=et_block(et, h), start=True, stop=True)
                pv4 = po.rearrange("p (a t q) -> p a t q", a=NCB, t=2)
                rr = pools["rr"].tile([P, 1024], f32, tag="rr", name="rr")
                rv4 = rr.rearrange("p (a t q) -> p a t q", a=NCB, t=2)
                # one full-tile recip at partition base 0: the custom DVE op
                # mishandles partition-base!=0 APs on hw (probe-verified);
                # non-denominator lanes produce junk that is never read.
                nc.vector.reciprocal_approx_fast(out=rr, in_=po)
                nc.vector.tensor_mul(
                    ot_full[0:DH, :, ssl], pv4[0:DH, :, 0, :], rv4[DH:P, :, 0, :])
                nc.vector.tensor_mul(
                    ot_full[DH:P, :, ssl], pv4[DH:P, :, 1, :], rv4[0:DH, :, 1, :])

            # scores run two seqs ahead of AV so exp (ACT) overlaps TensorE
            et_q = [scores_seq(0), scores_seq(1)]

            # V projections + scatter into the [V|1]/[1|V] head blocks
            # (fills TensorE while exp0/exp1 run)
            for sq in range(G):
                pv = pools["proj"].tile([P, C], f32, tag="proj", name="pv")
                if K_V8:
                    for j in range(NPR):
                        nc.tensor.matmul(
                            pv, lhsT=src_f8[j][:, :, s0 + sq, :], rhs=wv[j],
                            start=(j == 0), stop=(j == NPR - 1), perf_mode=DR)
                else:
                    for ci in range(NCB):
                        nc.tensor.matmul(
                            pv, lhsT=src_bf[ci][:, s0 + sq, :], rhs=wv[ci],
                            start=(ci == 0), stop=(ci == NCB - 1))
                vt = vt_bufs[vslot + sq]
                vv = vt.rearrange("p (a q d) -> p a q d", a=NCB, q=4)
                pvv = pv.rearrange("p (a t d) -> p a t d", a=NCB, t=2)
                copy_on(K_SCATTER_ENG, vv[:, :, 0, :], pvv[:, :, 0, :])
                copy_on(K_SCATTER_ENG, vv[:, :, 3, :], pvv[:, :, 1, :])

            for sq in range(G):
                if sq + 2 < G:
                    et_q.append(scores_seq(sq + 2))
                attn_tail(sq, et_q[sq])

            # next group's Q/K phase: PE fills with its projection matmuls
            # while ACT drains this group's exps, then its copies run during
            # our O-projection.
            nxt = next_qk() if next_qk is not None else None

            # --- O-projection (+ optional TensorE residual fold) ---
            pods = []
            for cop in range(NCB // 2):
                pp2 = pools["attn"].tile([P, 1024], f32, tag="attn", name="pp2")
                for half in range(2):
                    co = cop * 2 + half
                    dst = pp2[:, half * 512:(half + 1) * 512]
                    last = resid is None
                    if K_O8:
                        for j in range(NPR):
                            nc.tensor.matmul(
                                dst,
                                lhsT=wo[j][:, :, co * P:(co + 1) * P],
                                rhs=ot_full[:, 2 * j:2 * j + 2, :],
                                start=(j == 0), stop=last and (j == NPR - 1),
                                perf_mode=DR)
                    else:
                        for ci in range(NCB):
                            nc.tensor.matmul(
                                dst,
                                lhsT=wo[ci][:, co * P:(co + 1) * P],
                                rhs=ot_full[:, ci, :],
                                start=(ci == 0), stop=last and (ci == NCB - 1))
                    if resid is not None:
                        # fold the (xs + oh) residual into the O-proj psum on
                        # TensorE: += I.T @ resid_block
                        nc.tensor.matmul(dst, lhsT=ident, rhs=resid[co],
                                         start=False, stop=True)
                    pods.append(dst)
            return pods, nxt

        def load_src(pool, dram_bf, dram_f8, q0, ch):
            """load a seq chunk in the layouts the flags require."""
            src_bf, src_f8 = None, None
            if _NEED_BF_SRC:
                src_bf = []
                for cb in range(NCB):
                    t = pool.tile([P, ch, S], bf, tag=f"srcb{cb}", name=f"srcb{cb}")
                    nc.sync.dma_start(
                        out=t, in_=dram_bf[cb * P:(cb + 1) * P, q0:q0 + ch, :])
                    src_bf.append(t)
            if _NEED_F8_SRC:
                src_f8 = []
                for j in range(NPR):
                    t = pool.tile([P, 2, ch, S], f8, tag=f"src8{j}", name=f"src8{j}")
                    nc.sync.dma_start(
                        out=t,
                        in_=dram_f8[j * 2 * P:(j + 1) * 2 * P, q0:q0 + ch, :]
                        .rearrange("(i p) w h -> p i w h", i=2))
                    src_f8.append(t)
            return src_bf, src_f8

        def run_pass(kind):
            """kind: 'h' (height: seq along h, ohT2 scratch out) or
            'w' (width: seq along w, residual fold, natural out)."""
            if kind == "h":
                ch, nchunks = HC1, S // HC1
                wq, wk, wv, wo = (w_sb["wq_h"], w_sb["wk_h"],
                                  w_sb["wv_h"], w_sb["wo_h"])
                dram_bf, dram_f8 = xt, xt8
            else:
                ch, nchunks = HC2, S // HC2
                wq, wk, wv, wo = (w_sb["wq_w"], w_sb["wk_w"],
                                  w_sb["wv_w"], w_sb["wo_w"])
                dram_bf, dram_f8 = xn, xn8
            gpc = ch // G  # groups per chunk

            with ExitStack() as ctx:
                src_pool = ctx.enter_context(tc.tile_pool(name=f"src{kind}", bufs=2))
                stage_pool = ctx.enter_context(tc.tile_pool(name=f"stg{kind}", bufs=2))
                if kind == "w":
                    resid_pool = ctx.enter_context(tc.tile_pool(name="res2", bufs=2))
                    oh_pool = ctx.enter_context(tc.tile_pool(name="oh2", bufs=2))
                pools = mk_pools(ctx, kind)

                chunk_state = {}

                def get_chunk(c):
                    if c in chunk_state:
                        return chunk_state[c]
                    q0 = c * ch
                    src = load_src(src_pool, dram_bf, dram_f8, q0, ch)
                    stage_t, resid_t = [], []
                    for cb in range(NCB):
                        cs = slice(cb * P, (cb + 1) * P)
                        if kind == "h":
                            st = stage_pool.tile([P, HB, ch, HC2], bf,
                                                 tag=f"stg{cb}", name=f"stg{cb}")
                        else:
                            st = stage_pool.tile([P, ch, S], f32,
                                                 tag=f"stg{cb}", name=f"stg{cb}")
                            rt = resid_pool.tile([P, ch, S], f32,
                                                 tag=f"res{cb}", name=f"res{cb}")
                            nc.sync.dma_start(out=rt, in_=xf[cs, q0:q0 + ch, :])
                            oht = oh_pool.tile([P, S, ch], bf,
                                               tag=f"oh{cb}", name=f"oh{cb}")
                            nc.sync.dma_start(out=oht, in_=ohT2[c, cs, :, :])
                            # (xs + oh) -> bf16 on GpSimd; the bf16 rounding
                            # of the residual costs ~2e-3 rel, and the sum is
                            # then applied on TensorE via the identity matmul
                            rtb = resid_pool.tile([P, ch, S], bf,
                                                  tag=f"resb{cb}", name=f"resb{cb}")
                            nc.gpsimd.tensor_tensor(
                                out=rtb, in0=rt,
                                in1=oht.rearrange("p w i -> p i w"),
                                op=mybir.AluOpType.add)
                            resid_t.append(rtb)
                        stage_t.append(st)
                    chunk_state[c] = (src, stage_t, resid_t, q0)
                    return chunk_state[c]

                def finish_chunk(c):
                    src, stage_t, resid_t, q0 = chunk_state.pop(c)
                    for cb in range(NCB):
                        cs = slice(cb * P, (cb + 1) * P)
                        if kind == "h":
                            nc.sync.dma_start(
                                out=ohT2[:, cs, q0:q0 + ch, :].rearrange(
                                    "b c w i -> c b w i"),
                                in_=stage_t[cb])
                        else:
                            nc.sync.dma_start(out=out[cs, q0:q0 + ch, :],
                                              in_=stage_t[cb])

                groups = [(c, g) for c in range(nchunks) for g in range(gpc)]

                def make_qk(i):
                    c, g = groups[i]
                    src, _, _, _ = get_chunk(c)
                    return qk_phase(src[0], src[1],
                                    slice(g * G, g * G + G), wq, wk, pools)

                qtkt = make_qk(0)
                for i, (c, g) in enumerate(groups):
                    src, stage_t, resid_t, q0 = get_chunk(c)
                    s0 = g * G
                    gsl = slice(s0, s0 + G)
                    resid = ([resid_t[cb][:, gsl, :] for cb in range(NCB)]
                             if kind == "w" else None)
                    thunk = (lambda j=i + 1: make_qk(j)) if i + 1 < len(groups) else None
                    pods, qtkt = attn_body(qtkt, src[0], src[1], s0, wv, wo,
                                           pools, gen=i, resid=resid,
                                           next_qk=thunk)
                    for co in range(NCB):
                        if kind == "h":
                            # pods: (p, 4 w-seq, 128 h) -> stage (hb, w, hi)
                            copy_on(K_STGH_ENG,
                                    stage_t[co][:, :, gsl, :].rearrange(
                                        "p b q i -> p q b i"),
                                    pods[co].rearrange("p (q b i) -> p q b i",
                                                       q=G, b=HB))
                        else:
                            copy_on(K_STGW_ENG,
                                    stage_t[co][:, gsl, :],
                                    pods[co].rearrange("p (q s) -> p q s", q=G))
                    if g == gpc - 1:
                        finish_chunk(c)

        if reps == 1:
            run_pass("h")
            run_pass("w")
        else:
            with tc.For_i(0, reps):
                run_pass("h")
                run_pass("w")

    nc.compile()
    return nc


def _get_program(reps=1):
    if reps not in _PROG:
        _PROG[reps] = _build_program(reps)
    return _PROG[reps]


def _w_is_f8(n):
    if n.startswith("wo"):
        return K_O8
    if n.startswith("wv"):
        return K_V8
    return K_QK8


def _host_prep(xs, wmap):
    """per-core input maps from the full inputs (wmap in reference
    orientation (c_out, c_in); transposed + cast here)."""
    wt_np = {}
    for n, w in wmap.items():
        dt = _F8 if _w_is_f8(n) else _BF16
        wt_np[n] = np.ascontiguousarray(np.asarray(w, dtype=np.float32).T).astype(dt)

    xs = np.asarray(xs, dtype=np.float32)
    in_maps = []
    for b in range(NCORES):
        xb = np.ascontiguousarray(xs[b])                        # (C, H, W) f32
        xtb = np.ascontiguousarray(np.swapaxes(xb, 1, 2))       # (C, W, H) f32
        in_maps.append({
            "xf": xb,
            "xn": xb.astype(_BF16), "xt": xtb.astype(_BF16),
            "xn8": xb.astype(_F8), "xt8": xtb.astype(_F8),
            "idm": _IDM,
            **wt_np})
    return in_maps


def kernel(xs, Wq_h, Wk_h, Wv_h, Wo_h, Wq_w, Wk_w, Wv_w, Wo_w):
    from concourse.bass_utils import run_bass_kernel_spmd

    nc = _get_program()
    wmap = {
        "wq_w": Wq_w, "wk_w": Wk_w, "wv_w": Wv_w, "wo_w": Wo_w,
        "wq_h": Wq_h, "wk_h": Wk_h, "wv_h": Wv_h, "wo_h": Wo_h,
    }
    in_maps = _host_prep(xs, wmap)
    res = run_bass_kernel_spmd(nc, in_maps, core_ids=list(range(NCORES)))
    return np.stack([res.results[b]["out"] for b in range(NCORES)], axis=0)
